# revision 17
# baseline (speedup 1.0000x reference)
"""Trainium2 Bass kernel for nn_LogisticRegressionPerStock.

Math:  h = sigmoid(einsum("bts,st->bs", x, W1) + b1);  out = h @ W2.T + b2
Shapes: x [1024, 24, 8192], W1 [8192, 24], W2 [8192, 8192].

Sharding: stock dim S split across 8 cores (SLOC = 1024 each); each core
computes a full [B, S] partial of the final GEMM (contraction over its
local stocks); host sums the 8 bf16 partials and adds b2.

Per-core dataflow: W1 is folded into x on the host (x' = x * W1,
transposed to [S, B, T] fp8e4m3) so 128 stocks sit on SBUF partitions, t is
innermost, and the per-stock einsum is one DVE tensor_reduce (sum over t)
per (s-block, b-chunk); bias+sigmoid fuse into one ACT op writing hT
[s-part, b-free] bf16 — the GEMM's stationary layout, no PE transposes.
The [B,S] GEMM runs in bf16 (1 cycle/row), hT slices stationary (4
consecutive 512-wide matmuls per weight load), accumulating over the 8
local k-tiles in PSUM.  B is pipelined in chunks of 128: the reduce of
chunk c+1 overlaps the GEMM of chunk c, and within chunk 0 the GEMM's
k-accumulation consumes each s-block as its reduce lands.  Engine/queue
split: x + W2 loads on the SP queue (W2 streams as 8 column-pieces
behind the early x tiles); sigmoids, PSUM->SBUF copies and output stores
on the ACT engine/queue so stores never block the x stream.
"""

import sys

sys.path.insert(0, "/opt/trn_rl_repo")

import numpy as np
import ml_dtypes

B, T, S = 1024, 24, 8192
NCORES = 8
SLOC = S // NCORES  # 1024 stocks per core
P = 128
NSB = SLOC // P  # 8 s-blocks = GEMM k-tiles per core
BC = 128  # b-chunk size for the E/G pipeline
NBC = B // BC  # 8 chunks
NG = 4  # output column groups per chunk (2048 cols each)
GW = S // NG  # 2048
NPS = GW // 512  # 4 psum tiles per group

_compiled = {}


def _build_nc():
    import concourse.bass as bass
    import concourse.bacc as bacc
    import concourse.tile as tile
    from concourse import mybir

    f32 = mybir.dt.float32
    f16 = mybir.dt.float16
    f8 = mybir.dt.float8e4
    bf16 = mybir.dt.bfloat16
    ADD = mybir.AluOpType.add

    nc = bacc.Bacc()
    # xw: [S_loc, B, T] fp16, xw[s, b, t] = x[b, t, s] * W1[s, t]
    xt_d = nc.dram_tensor("xw", [SLOC, B, T], f8, kind="ExternalInput")
    # b1e: [128, NSB]; [p, k] = b1[k*128+p]
    b1_d = nc.dram_tensor("b1e", [P, NSB], f32, kind="ExternalInput")
    # w2t: [128, NSB, S]; [p, k, n] = W2.T[k*128+p, n] for this core's rows
    w2_d = nc.dram_tensor("w2t", [P, NSB, S], bf16, kind="ExternalInput")
    out_d = nc.dram_tensor("part", [B, S], bf16, kind="ExternalOutput")

    with tile.TileContext(nc) as tc:
        with (
            tc.tile_pool(name="persist", bufs=1) as pp,
            # bufs=8: e_dma(c+1) issues all 8 x DMAs before e_compute(c+1)
            # is traced, so each DMA must land in a slot whose previous
            # occupant's reader (chunk c's reduce) is already traced
            tc.tile_pool(name="xp", bufs=8) as xp,
            tc.tile_pool(name="ep", bufs=4) as ep,
            tc.tile_pool(name="stp", bufs=2) as stp,
            tc.tile_pool(name="psp", bufs=8, space="PSUM") as psp,
        ):
            b1sb = pp.tile([P, NSB], f32, tag="b1sb")
            nc.sync.dma_start(b1sb[:], b1_d[:, :])
            w2sb = pp.tile([P, NSB, S], bf16, tag="w2sb")
            hts = [
                pp.tile([P, NSB, BC], bf16, name=f"ht{c}", tag=f"ht{c}")
                for c in range(NBC)
            ]
            xtiles = {}

            PW = S // NSB  # 1024-col W2 stream piece

            def e_dma(c):
                for sb in range(NSB):
                    xtile = xp.tile([P, BC, T], f8, tag="xt", name="xtile")
                    nc.sync.dma_start(
                        xtile[:],
                        xt_d[sb * P : (sb + 1) * P, c * BC : (c + 1) * BC, :],
                    )
                    xtiles[(c, sb)] = xtile
                    # W2 column-pieces stream behind the first x tiles; the
                    # GEMM's group-g matmuls block on their arrival via
                    # subtile RAW deps
                    if c == 0 and sb % 2 == 1:
                        for j in (sb - 1, sb):
                            nc.sync.dma_start(
                                w2sb[:, :, j * PW : (j + 1) * PW],
                                w2_d[:, :, j * PW : (j + 1) * PW],
                            )

            def e_compute(c):
                for sb in range(NSB):
                    acc = ep.tile([P, BC], f16, tag="acc")
                    with nc.allow_low_precision("fp16 sum of 24 terms, validated"):
                        nc.vector.tensor_reduce(
                            acc[:], xtiles[(c, sb)][:], mybir.AxisListType.X, ADD
                        )
                    nc.scalar.activation(
                        hts[c][:, sb, :],
                        acc[:],
                        mybir.ActivationFunctionType.Sigmoid,
                        bias=b1sb[:, sb : sb + 1],
                    )

            def phase_g(c):
                bt = c  # BC == P: one b-tile per chunk
                for g in range(NG):
                    pss = [
                        psp.tile([P, 512], f32, tag="ps", name=f"ps{n}")
                        for n in range(NPS)
                    ]
                    for k in range(NSB):
                        lhsT = hts[c][:, k, :]
                        for n in range(NPS):
                            nc.tensor.matmul(
                                pss[n][:],
                                lhsT,
                                w2sb[:, k, g * GW + n * 512 : g * GW + (n + 1) * 512],
                                start=(k == 0),
                                stop=(k == NSB - 1),
                            )
                    stg = stp.tile([P, GW], bf16, tag="stg", name="stg")
                    for n in range(NPS):
                        nc.scalar.activation(
                            stg[:, n * 512 : (n + 1) * 512],
                            pss[n][:],
                            mybir.ActivationFunctionType.Copy,
                        )
                    # out stores ride the ACT queue so they never block the
                    # x/W2 stream on the sync queue
                    nc.scalar.dma_start(
                        out_d[bt * P : (bt + 1) * P, g * GW : (g + 1) * GW],
                        stg[:],
                    )

            e_dma(0)
            e_compute(0)
            for c in range(NBC):
                if c + 1 < NBC:
                    e_dma(c + 1)
                phase_g(c)
                if c + 1 < NBC:
                    e_compute(c + 1)
    nc.finalize()
    return nc


def _get_nc():
    if "nc" not in _compiled:
        _compiled["nc"] = _build_nc()
    return _compiled["nc"]


def _host_prep(x, W1, b1, W2):
    # xw[s, b, t] = x[b, t, s] * W1[s, t], fp16
    xw = np.ascontiguousarray(x.transpose(2, 0, 1))  # [S, B, T]
    xw *= W1[:, None, :]
    xw = xw.astype(ml_dtypes.float8_e4m3)
    W2T = W2.T.astype(ml_dtypes.bfloat16)  # [S_in, S_out]
    b1e = b1.astype(np.float32)
    in_maps = []
    for k in range(NCORES):
        sk = slice(k * SLOC, (k + 1) * SLOC)
        in_maps.append(
            {
                "xw": xw[sk],
                # [P, NSB] / [P, NSB, S] to match the SBUF tile dim order
                "b1e": np.ascontiguousarray(b1e[sk].reshape(NSB, P).T),
                "w2t": np.ascontiguousarray(
                    W2T[sk].reshape(NSB, P, S).transpose(1, 0, 2)
                ),
            }
        )
    return in_maps


def kernel(x, W1, b1, W2, b2):
    from concourse.bass_utils import run_bass_kernel_spmd

    nc = _get_nc()
    in_maps = _host_prep(
        np.asarray(x, dtype=np.float32),
        np.asarray(W1, dtype=np.float32),
        np.asarray(b1, dtype=np.float32),
        np.asarray(W2, dtype=np.float32),
    )
    res = run_bass_kernel_spmd(nc, in_maps, list(range(NCORES)))
    out = np.zeros((B, S), dtype=np.float32)
    for k in range(NCORES):
        out += res.results[k]["part"].astype(np.float32)
    out += np.asarray(b2, dtype=np.float32)[None, :]
    return out
